# revision 5
# baseline (speedup 1.0000x reference)
"""DiffPool encoder kernel for Trainium2 (Bass/Tile), 8-core SPMD.

Problem (hardcoded shapes):
  S [12288, 10] f32 assignment logits, A [12288, 12288] f32 adjacency,
  X [12288, 300] f32 features, idx [12288] i64 (sorted graph ids),
  n () i64 = 32 nodes/graph. 384 graphs.

  out0 X_cat [3840, 300] = concat_g softmax(S_g)^T X_g
  out1 A_bd [3840, 3840] = block_diag_g softmax(S_g)^T A_g softmax(S_g)

Sharding: graphs split across 8 cores (48 graphs each). Pooling is
block-diagonal per graph, so each core only needs its rows of S/X and the
48 diagonal 32x32 blocks of A. Per core, graphs are processed in 12 groups
of 4 (4*32 = 128 nodes = full partition dim):
  - softmax over K=10 for all 1536 rows (one exp + segmented reduce)
  - SD   [128, 12*40]: per group a block-diag [128,40] of normalized S
  - BD   [128, 12*128]: per group a block-diag [128,128] of A_b^T scaled
         by the softmax row reciprocals (folds normalization into A path)
  - mm2: T = BD_g^T @ E_g          -> A_b @ softmax(S_b) rows   [128,10]
  - mm1: XO = SD_g^T @ X_g                                      [40,300]
  - mm3: AO = SD_g^T @ T_g                                      [40,10]
Host extracts the A diagonal blocks (transposed per block) before launch
and scatters AO blocks into the block-diagonal output after.
"""

import os
import numpy as np
from contextlib import ExitStack

B = 384        # graphs
NPER = 32      # nodes per graph
K = 10         # clusters
D = 300        # feature dim
NCORES = 8
GPC = B // NCORES          # 48 graphs per core
GRP = 4                    # graphs per 128-row group
NG = GPC // GRP            # 12 groups per core
ROWS = GPC * NPER          # 1536 node rows per core

_CACHE = {}
LAST_RESULTS = None        # BassKernelResults of the most recent run


def _body(ctx, tc, S_d, X_d, AT_d, XO_d, AO_d):
    import concourse.bass as bass
    import concourse.mybir as mybir

    nc = tc.nc
    f32 = mybir.dt.float32

    const = ctx.enter_context(tc.tile_pool(name="const", bufs=1))
    xpool = ctx.enter_context(tc.tile_pool(name="xin", bufs=4))
    psx = ctx.enter_context(tc.tile_pool(name="psx", bufs=2, space="PSUM"))
    psa = ctx.enter_context(tc.tile_pool(name="psa", bufs=2, space="PSUM"))
    pst = ctx.enter_context(tc.tile_pool(name="pst", bufs=2, space="PSUM"))

    S_t = const.tile([128, NG * K], f32, tag="S_t")   # S[p, gK+k] = S_shard[128g+p, k]
    E = const.tile([128, NG * K], f32, tag="E")       # exp(S)
    Sn = const.tile([128, NG * K], f32, tag="Sn")     # softmax rows
    sums = const.tile([128, NG], f32, tag="sums")
    rinv = const.tile([128, NG], f32, tag="rinv")
    Ast = const.tile([128, NG * NPER], f32, tag="Ast")  # [32b+q, 32g+p] = A_b[p, q]
    Asc = const.tile([128, NG * NPER], f32, tag="Asc")  # Ast * rinv
    SD = const.tile([128, NG * GRP * K], f32, tag="SD")   # 12 block-diag [128,40]
    BD = const.tile([128, NG * 128], f32, tag="BD")       # 12 block-diag [128,128]
    Tsb = const.tile([128, NG * K], f32, tag="Tsb")   # T = A_b @ softmax(S_b)
    XOs = const.tile([GRP * K, NG * D], f32, tag="XOs")   # X_out staging
    AOs = const.tile([GRP * K, NG * K], f32, tag="AOs")   # A_out staging

    # ---- inputs ----
    nc.sync.dma_start(
        S_t[:].rearrange("p (g k) -> p g k", k=K),
        S_d.rearrange("(g p) k -> p g k", p=128),
    )
    nc.sync.dma_start(
        Ast[:].rearrange("r (g p) -> r g p", p=NPER),
        AT_d.rearrange("(g r) p -> r g p", r=128),
    )

    # ---- softmax over K within each group column block ----
    nc.scalar.activation(E[:], S_t[:], mybir.ActivationFunctionType.Exp)
    E3 = E[:].rearrange("p (g k) -> p g k", k=K)
    nc.vector.reduce_sum(sums[:], E3, axis=mybir.AxisListType.X)
    nc.vector.reciprocal(rinv[:], sums[:])
    rb = rinv[:].unsqueeze(2)
    nc.vector.tensor_mul(Sn[:].rearrange("p (g k) -> p g k", k=K), E3,
                         rb.broadcast_to([128, NG, K]))
    A3 = Ast[:].rearrange("p (g q) -> p g q", q=NPER)
    nc.vector.tensor_mul(Asc[:].rearrange("p (g q) -> p g q", q=NPER), A3,
                         rb.broadcast_to([128, NG, NPER]))

    # ---- block-diagonal placement (one copy per sub-block row b, all groups) ----
    nc.vector.memset(SD[:], 0.0)
    nc.vector.memset(BD[:], 0.0)
    Sn3 = Sn[:].rearrange("p (g k) -> p g k", k=K)
    Asc3 = Asc[:].rearrange("p (g q) -> p g q", q=NPER)
    SDv = SD[:].rearrange("p (g m) -> p g m", m=GRP * K)
    BDv = BD[:].rearrange("p (g m) -> p g m", m=128)
    for b in range(GRP):
        ps = slice(NPER * b, NPER * (b + 1))
        nc.vector.tensor_copy(SDv[ps, :, K * b:K * (b + 1)], Sn3[ps, :, :])
        nc.vector.tensor_copy(BDv[ps, :, NPER * b:NPER * (b + 1)], Asc3[ps, :, :])

    # ---- per group: X DMA + 3 matmuls + staging copies ----
    for g in range(NG):
        xt = xpool.tile([128, D], f32)
        nc.sync.dma_start(xt[:], X_d[128 * g:128 * (g + 1), :])

        tp = pst.tile([128, K], f32)
        nc.tensor.matmul(tp[:], BD[:, 128 * g:128 * (g + 1)],
                         E[:, K * g:K * (g + 1)], start=True, stop=True)
        nc.vector.tensor_copy(Tsb[:, K * g:K * (g + 1)], tp[:])

        xo = psx.tile([GRP * K, D], f32)
        nc.tensor.matmul(xo[:], SD[:, GRP * K * g:GRP * K * (g + 1)], xt[:],
                         start=True, stop=True)
        nc.scalar.copy(XOs[:, D * g:D * (g + 1)], xo[:])
        nc.sync.dma_start(XO_d[GRP * K * g:GRP * K * (g + 1), :],
                          XOs[:, D * g:D * (g + 1)])

        ao = psa.tile([GRP * K, K], f32)
        nc.tensor.matmul(ao[:], SD[:, GRP * K * g:GRP * K * (g + 1)],
                         Tsb[:, K * g:K * (g + 1)], start=True, stop=True)
        nc.vector.tensor_copy(AOs[:, K * g:K * (g + 1)], ao[:])

    nc.sync.dma_start(
        AO_d.rearrange("(g r) k -> r g k", r=GRP * K),
        AOs[:].rearrange("r (g k) -> r g k", k=K),
    )


def _build():
    if "nc" in _CACHE:
        return _CACHE["nc"]
    import concourse.bacc as bacc
    import concourse.tile as tile
    import concourse.mybir as mybir

    f32 = mybir.dt.float32
    nc = bacc.Bacc("TRN2", target_bir_lowering=False, debug=False)
    S_d = nc.dram_tensor("S", [ROWS, K], f32, kind="ExternalInput").ap()
    X_d = nc.dram_tensor("X", [ROWS, D], f32, kind="ExternalInput").ap()
    AT_d = nc.dram_tensor("AT", [ROWS, NPER], f32, kind="ExternalInput").ap()
    XO_d = nc.dram_tensor("XO", [GPC * K, D], f32, kind="ExternalOutput").ap()
    AO_d = nc.dram_tensor("AO", [GPC * K, K], f32, kind="ExternalOutput").ap()

    with tile.TileContext(nc) as tc:
        with ExitStack() as ctx:
            _body(ctx, tc, S_d, X_d, AT_d, XO_d, AO_d)
    nc.compile()
    _CACHE["nc"] = nc
    return nc


def kernel(S, A, X, idx=None, n=NPER, **_):
    global LAST_RESULTS
    from concourse.bass_utils import run_bass_kernel_spmd

    S = np.ascontiguousarray(np.asarray(S, dtype=np.float32))
    A = np.asarray(A, dtype=np.float32)
    X = np.ascontiguousarray(np.asarray(X, dtype=np.float32))
    n = int(np.asarray(n)) if n is not None else NPER
    assert n == NPER and S.shape == (B * NPER, K) and X.shape == (B * NPER, D)

    # Diagonal 32x32 blocks of A, transposed per block: AT[32j+q, p] = A_j[p, q]
    bi = np.arange(B)
    blocks = A.reshape(B, NPER, B, NPER)[bi, :, bi, :]        # [384, 32, 32]
    AT = np.ascontiguousarray(blocks.transpose(0, 2, 1)).reshape(B * NPER, NPER)

    S8 = S.reshape(NCORES, ROWS, K)
    X8 = X.reshape(NCORES, ROWS, D)
    AT8 = AT.reshape(NCORES, ROWS, NPER)
    in_maps = [{"S": S8[c], "X": X8[c], "AT": AT8[c]} for c in range(NCORES)]

    nc = _build()
    kw = {}
    if os.environ.get("KERNEL_TRACE"):
        kw = dict(trace=True, tmpdir=os.environ.get("KERNEL_TRACE_DIR") or None)
    res = run_bass_kernel_spmd(nc, in_maps, list(range(NCORES)), **kw)
    LAST_RESULTS = res

    X_cat = np.concatenate([r["XO"] for r in res.results], axis=0)   # [3840,300]
    AO = np.stack([r["AO"] for r in res.results]).reshape(B, K, K)
    A_bd = np.zeros((B * K, B * K), dtype=np.float32)
    A_bd.reshape(B, K, B, K)[bi, :, bi, :] = AO
    return X_cat, A_bd


# revision 10
# speedup vs baseline: 1.3743x; 1.3743x over previous
"""DiffPool encoder kernel for Trainium2 (Bass/Tile), 8-core SPMD.

Problem (hardcoded shapes):
  S [12288, 10] f32 assignment logits, A [12288, 12288] f32 adjacency,
  X [12288, 300] f32 features, idx [12288] i64 (sorted graph ids),
  n () i64 = 32 nodes/graph. 384 graphs.

  out0 X_cat [3840, 300] = concat_g softmax(S_g)^T X_g
  out1 A_bd [3840, 3840] = block_diag_g softmax(S_g)^T A_g softmax(S_g)

Sharding: graphs split across 8 cores (48 graphs each). Pooling is
block-diagonal per graph, so each core only needs its rows of S/X and the
48 diagonal 32x32 blocks of A. Per core, graphs are processed in 12 groups
of 4 (4*32 = 128 nodes = full partition dim):
  - softmax over K=10 for all 1536 rows (one exp + segmented reduce)
  - SD [128, 12*40]: per group a block-diag [128,40] of normalized S
  - BD [128, 12*128]: per group a block-diag [128,128] of A_b^T
  - mm2: T  = BD_g^T @ Sn_g = A_b @ softmax(S_b), stacked    [128,10]
  - mm1: XO = SD_g^T @ X_g                                   [40,300]
  - mm3: AO = SD_g^T @ T_g                                   [40,10]
All matmuls run as float32r (single-pass fp32 on the PE).
Host extracts the A diagonal blocks (transposed per block) before launch
and scatters AO blocks into the block-diagonal output after.

DMA routing: sync HWDGE ring streams inputs (AT + X in 4 chunks of 3
groups), scalar HWDGE ring carries S and all stores (XO in 4 chunks, AO),
so dependency-gated stores never head-of-line-block the input stream.
SBUF->SBUF block placements run on GpSimd; PSUM->SBUF evacuation
alternates between Scalar and Vector.
"""

import os
import numpy as np
from contextlib import ExitStack

B = 384        # graphs
NPER = 32      # nodes per graph
K = 10         # clusters
D = 300        # feature dim
NCORES = 8
GPC = B // NCORES          # 48 graphs per core
GRP = 4                    # graphs per 128-row group
NG = GPC // GRP            # 12 groups per core
ROWS = GPC * NPER          # 1536 node rows per core
XCH = 3                    # groups per X input DMA chunk
OCH = 3                    # groups per XO output DMA chunk

_CACHE = {}
LAST_RESULTS = None        # BassKernelResults of the most recent run


def _body(ctx, tc, S_d, X_d, AT_d, XO_d, AO_d):
    import concourse.bass as bass
    import concourse.mybir as mybir

    nc = tc.nc
    f32 = mybir.dt.float32
    f32r = mybir.dt.float32r

    const = ctx.enter_context(tc.tile_pool(name="const", bufs=1))
    xpool = ctx.enter_context(tc.tile_pool(name="xin", bufs=2))
    psx = ctx.enter_context(tc.tile_pool(name="psx", bufs=2, space="PSUM"))
    psa = ctx.enter_context(tc.tile_pool(name="psa", bufs=2, space="PSUM"))
    pst = ctx.enter_context(tc.tile_pool(name="pst", bufs=2, space="PSUM"))

    S_t = const.tile([128, NG * K], f32, tag="S_t")   # S[p, gK+k] = S_shard[128g+p, k]
    E = const.tile([128, NG * K], f32, tag="E")       # exp(S)
    Sn = const.tile([128, NG * K], f32r, tag="Sn")    # softmax rows (f32r for PE)
    sums = const.tile([128, NG], f32, tag="sums")
    rinv = const.tile([128, NG], f32, tag="rinv")
    Ast = const.tile([128, NG * NPER], f32, tag="Ast")  # [32b+q, 32g+p] = A_b[p, q]
    SD = const.tile([128, NG * GRP * K], f32r, tag="SD")  # 12 block-diag [128,40]
    BD = const.tile([128, NG * 128], f32r, tag="BD")      # 12 block-diag [128,128]
    Tsb = const.tile([128, NG * K], f32r, tag="Tsb")  # T = A_b @ softmax(S_b)
    XOs = const.tile([GRP * K, NG * D], f32, tag="XOs")   # X_out staging
    AOs = const.tile([GRP * K, NG * K], f32, tag="AOs")   # A_out staging

    # ---- inputs: S on the scalar (ACT) ring, AT + X chunks on sync ----
    nc.scalar.dma_start(
        S_t[:].rearrange("p (g k) -> p g k", k=K),
        S_d.rearrange("(g p) k -> p g k", p=128),
    )
    nc.sync.dma_start(
        Ast[:].rearrange("r (g p) -> r g p", p=NPER),
        AT_d.rearrange("(g r) p -> r g p", r=128),
    )
    xts = []
    for c in range(NG // XCH):
        xt = xpool.tile([128, XCH * D], f32r)
        nc.sync.dma_start(
            xt[:].rearrange("p (g d) -> p g d", d=D),
            X_d[128 * XCH * c:128 * XCH * (c + 1), :].rearrange(
                "(g p) d -> p g d", p=128),
        )
        xts.append(xt)

    # ---- softmax over K within each group column block ----
    nc.scalar.activation(E[:], S_t[:], mybir.ActivationFunctionType.Exp)
    E3 = E[:].rearrange("p (g k) -> p g k", k=K)
    nc.vector.reduce_sum(sums[:], E3, axis=mybir.AxisListType.X)
    nc.vector.reciprocal(rinv[:], sums[:])
    rb = rinv[:].unsqueeze(2)
    nc.vector.tensor_mul(Sn[:].rearrange("p (g k) -> p g k", k=K), E3,
                         rb.broadcast_to([128, NG, K]))

    # ---- block-diagonal placement (SBUF->SBUF) ----
    nc.vector.memset(SD[:].bitcast(f32), 0.0)
    nc.vector.memset(BD[:].bitcast(f32), 0.0)
    Sn3 = Sn[:].rearrange("p (g k) -> p g k", k=K)
    A3 = Ast[:].rearrange("p (g q) -> p g q", q=NPER)
    SDv = SD[:].rearrange("p (g m) -> p g m", m=GRP * K)
    BDv = BD[:].rearrange("p (g m) -> p g m", m=128)
    for b in range(GRP):
        ps = slice(NPER * b, NPER * (b + 1))
        nc.vector.tensor_copy(SDv[ps, :, K * b:K * (b + 1)], Sn3[ps, :, :])
        nc.vector.tensor_copy(BDv[ps, :, NPER * b:NPER * (b + 1)], A3[ps, :, :])

    # ---- per group: 3 matmuls (float32r) + staged evacuation ----
    for g in range(NG):
        # PSUM evacuation engines alternate between Scalar (ACT) and Vector
        def cp_a(out, in_, even=(g % 2 == 0)):
            (nc.scalar.copy if even else nc.vector.tensor_copy)(out, in_)

        def cp_b(out, in_, even=(g % 2 == 0)):
            (nc.vector.tensor_copy if even else nc.scalar.copy)(out, in_)

        tp = pst.tile([128, K], f32)
        nc.tensor.matmul(tp[:], BD[:, 128 * g:128 * (g + 1)],
                         Sn[:, K * g:K * (g + 1)],
                         start=True, stop=True)
        cp_b(Tsb[:, K * g:K * (g + 1)], tp[:])

        xt = xts[g // XCH]
        xo = psx.tile([GRP * K, D], f32)
        nc.tensor.matmul(xo[:], SD[:, GRP * K * g:GRP * K * (g + 1)],
                         xt[:, D * (g % XCH):D * (g % XCH + 1)],
                         start=True, stop=True)
        cp_a(XOs[:, D * g:D * (g + 1)], xo[:])

        ao = psa.tile([GRP * K, K], f32)
        nc.tensor.matmul(ao[:], SD[:, GRP * K * g:GRP * K * (g + 1)],
                         Tsb[:, K * g:K * (g + 1)],
                         start=True, stop=True)
        cp_b(AOs[:, K * g:K * (g + 1)], ao[:])

        # ship finished XO chunks on the scalar ring
        if g % OCH == OCH - 1:
            c = g // OCH
            nc.scalar.dma_start(
                XO_d[GRP * K * OCH * c:GRP * K * OCH * (c + 1), :].rearrange(
                    "(g r) d -> r g d", r=GRP * K),
                XOs[:, D * OCH * c:D * OCH * (c + 1)].rearrange(
                    "r (g d) -> r g d", d=D),
            )

    nc.scalar.dma_start(
        AO_d.rearrange("(g r) k -> r g k", r=GRP * K),
        AOs[:].rearrange("r (g k) -> r g k", k=K),
    )


def _build():
    if "nc" in _CACHE:
        return _CACHE["nc"]
    import concourse.bacc as bacc
    import concourse.tile as tile
    import concourse.mybir as mybir

    f32 = mybir.dt.float32
    nc = bacc.Bacc("TRN2", target_bir_lowering=False, debug=False)
    S_d = nc.dram_tensor("S", [ROWS, K], f32, kind="ExternalInput").ap()
    X_d = nc.dram_tensor("X", [ROWS, D], mybir.dt.float32r, kind="ExternalInput").ap()
    AT_d = nc.dram_tensor("AT", [ROWS, NPER], f32, kind="ExternalInput").ap()
    XO_d = nc.dram_tensor("XO", [GPC * K, D], f32, kind="ExternalOutput").ap()
    AO_d = nc.dram_tensor("AO", [GPC * K, K], f32, kind="ExternalOutput").ap()

    with tile.TileContext(nc) as tc:
        with ExitStack() as ctx:
            _body(ctx, tc, S_d, X_d, AT_d, XO_d, AO_d)
    nc.compile()
    _CACHE["nc"] = nc
    return nc


def kernel(S, A, X, idx=None, n=NPER, **_):
    global LAST_RESULTS
    from concourse.bass_utils import run_bass_kernel_spmd

    S = np.ascontiguousarray(np.asarray(S, dtype=np.float32))
    A = np.asarray(A, dtype=np.float32)
    X = np.ascontiguousarray(np.asarray(X, dtype=np.float32))
    n = int(np.asarray(n)) if n is not None else NPER
    assert n == NPER and S.shape == (B * NPER, K) and X.shape == (B * NPER, D)

    # Diagonal 32x32 blocks of A, transposed per block: AT[32j+q, p] = A_j[p, q]
    bi = np.arange(B)
    blocks = A.reshape(B, NPER, B, NPER)[bi, :, bi, :]        # [384, 32, 32]
    AT = np.ascontiguousarray(blocks.transpose(0, 2, 1)).reshape(B * NPER, NPER)

    S8 = S.reshape(NCORES, ROWS, K)
    X8 = X.reshape(NCORES, ROWS, D)
    AT8 = AT.reshape(NCORES, ROWS, NPER)
    in_maps = [{"S": S8[c], "X": X8[c], "AT": AT8[c]} for c in range(NCORES)]

    nc = _build()
    kw = {}
    if os.environ.get("KERNEL_TRACE"):
        kw = dict(trace=True, tmpdir=os.environ.get("KERNEL_TRACE_DIR") or None)
    res = run_bass_kernel_spmd(nc, in_maps, list(range(NCORES)), **kw)
    LAST_RESULTS = res

    X_cat = np.concatenate([r["XO"] for r in res.results], axis=0)   # [3840,300]
    AO = np.stack([r["AO"] for r in res.results]).reshape(B, K, K)
    A_bd = np.zeros((B * K, B * K), dtype=np.float32)
    A_bd.reshape(B, K, B, K)[bi, :, bi, :] = AO
    return X_cat, A_bd


# revision 11
# speedup vs baseline: 1.3848x; 1.0076x over previous
"""DiffPool encoder kernel for Trainium2 (Bass/Tile), 8-core SPMD.

Problem (hardcoded shapes):
  S [12288, 10] f32 assignment logits, A [12288, 12288] f32 adjacency,
  X [12288, 300] f32 features, idx [12288] i64 (sorted graph ids),
  n () i64 = 32 nodes/graph. 384 graphs.

  out0 X_cat [3840, 300] = concat_g softmax(S_g)^T X_g
  out1 A_bd [3840, 3840] = block_diag_g softmax(S_g)^T A_g softmax(S_g)

Sharding: graphs split across 8 cores (48 graphs each). Pooling is
block-diagonal per graph, so each core only needs its rows of S/X and the
48 diagonal 32x32 blocks of A. Per core, graphs are processed in 12 groups
of 4 (4*32 = 128 nodes = full partition dim):
  - softmax over K=10 for all 1536 rows (one exp + segmented reduce)
  - SD [128, 12*40]: per group a block-diag [128,40] of normalized S
  - BD [128, 12*128]: per group a block-diag [128,128] of A_b^T
  - mm2: T  = BD_g^T @ Sn_g = A_b @ softmax(S_b), stacked    [128,10]
  - mm1: XO = SD_g^T @ X_g                                   [40,300]
  - mm3: AO = SD_g^T @ T_g                                   [40,10]
All matmuls run as float32r (single-pass fp32 on the PE).
Host extracts the A diagonal blocks (transposed per block) before launch
and scatters AO blocks into the block-diagonal output after.

DMA routing: sync HWDGE ring streams inputs (AT + X in 4 chunks of 3
groups), scalar HWDGE ring carries S and all stores (XO in 4 chunks, AO),
so dependency-gated stores never head-of-line-block the input stream.
SBUF->SBUF block placements run on GpSimd; PSUM->SBUF evacuation
alternates between Scalar and Vector.
"""

import os
import numpy as np
from contextlib import ExitStack

B = 384        # graphs
NPER = 32      # nodes per graph
K = 10         # clusters
D = 300        # feature dim
NCORES = 8
GPC = B // NCORES          # 48 graphs per core
GRP = 4                    # graphs per 128-row group
NG = GPC // GRP            # 12 groups per core
ROWS = GPC * NPER          # 1536 node rows per core
XCH = 3                    # groups per X input DMA chunk
OCH = 3                    # groups per XO output DMA chunk

_CACHE = {}
LAST_RESULTS = None        # BassKernelResults of the most recent run


def _body(ctx, tc, S_d, X_d, AT_d, XO_d, AO_d):
    import concourse.bass as bass
    import concourse.mybir as mybir

    nc = tc.nc
    f32 = mybir.dt.float32
    f32r = mybir.dt.float32r

    const = ctx.enter_context(tc.tile_pool(name="const", bufs=1))
    xpool = ctx.enter_context(tc.tile_pool(name="xin", bufs=2))
    psx = ctx.enter_context(tc.tile_pool(name="psx", bufs=2, space="PSUM"))
    psa = ctx.enter_context(tc.tile_pool(name="psa", bufs=2, space="PSUM"))
    pst = ctx.enter_context(tc.tile_pool(name="pst", bufs=2, space="PSUM"))

    S_t = const.tile([128, NG * K], f32, tag="S_t")   # S[p, gK+k] = S_shard[128g+p, k]
    E = const.tile([128, NG * K], f32, tag="E")       # exp(S)
    Sn = const.tile([128, NG * K], f32r, tag="Sn")    # softmax rows (f32r for PE)
    sums = const.tile([128, NG], f32, tag="sums")
    rinv = const.tile([128, NG], f32, tag="rinv")
    Ast = const.tile([128, NG * NPER], f32, tag="Ast")  # [32b+q, 32g+p] = A_b[p, q]
    SD = const.tile([128, NG * GRP * K], f32r, tag="SD")  # 12 block-diag [128,40]
    BD = const.tile([128, NG * 128], f32r, tag="BD")      # 12 block-diag [128,128]
    Tsb = const.tile([128, NG * K], f32r, tag="Tsb")  # T = A_b @ softmax(S_b)
    XOs = const.tile([GRP * K, NG * D], f32, tag="XOs")   # X_out staging
    AOs = const.tile([GRP * K, NG * K], f32, tag="AOs")   # A_out staging

    # ---- zero-fills first so they never block the softmax chain ----
    nc.vector.memset(SD[:].bitcast(f32), 0.0)
    nc.vector.memset(BD[:].bitcast(f32), 0.0)

    # ---- inputs: S first on the scalar (ACT) ring, AT on sync,
    # X chunks alternate between the two HWDGE rings ----
    nc.scalar.dma_start(
        S_t[:].rearrange("p (g k) -> p g k", k=K),
        S_d.rearrange("(g p) k -> p g k", p=128),
    )
    nc.sync.dma_start(
        Ast[:].rearrange("r (g p) -> r g p", p=NPER),
        AT_d.rearrange("(g r) p -> r g p", r=128),
    )
    xts = []
    for c in range(NG // XCH):
        xt = xpool.tile([128, XCH * D], f32r)
        (nc.sync if c % 2 == 0 else nc.scalar).dma_start(
            xt[:].rearrange("p (g d) -> p g d", d=D),
            X_d[128 * XCH * c:128 * XCH * (c + 1), :].rearrange(
                "(g p) d -> p g d", p=128),
        )
        xts.append(xt)

    # ---- softmax over K within each group column block ----
    nc.scalar.activation(E[:], S_t[:], mybir.ActivationFunctionType.Exp)
    E3 = E[:].rearrange("p (g k) -> p g k", k=K)
    nc.vector.reduce_sum(sums[:], E3, axis=mybir.AxisListType.X)
    nc.vector.reciprocal(rinv[:], sums[:])
    rb = rinv[:].unsqueeze(2)
    nc.vector.tensor_mul(Sn[:].rearrange("p (g k) -> p g k", k=K), E3,
                         rb.broadcast_to([128, NG, K]))

    # ---- block-diagonal placement: SD on DVE (after softmax), BD on ACT ----
    Sn3 = Sn[:].rearrange("p (g k) -> p g k", k=K)
    A3 = Ast[:].rearrange("p (g q) -> p g q", q=NPER)
    SDv = SD[:].rearrange("p (g m) -> p g m", m=GRP * K)
    BDv = BD[:].rearrange("p (g m) -> p g m", m=128)
    for b in range(GRP):
        ps = slice(NPER * b, NPER * (b + 1))
        nc.vector.tensor_copy(SDv[ps, :, K * b:K * (b + 1)], Sn3[ps, :, :])
        nc.scalar.copy(BDv[ps, :, NPER * b:NPER * (b + 1)], A3[ps, :, :])

    # ---- per group: 3 matmuls (float32r) + staged evacuation ----
    for g in range(NG):
        # PSUM evacuation engines alternate between Scalar (ACT) and Vector
        def cp_a(out, in_, even=(g % 2 == 0)):
            (nc.scalar.copy if even else nc.vector.tensor_copy)(out, in_)

        def cp_b(out, in_, even=(g % 2 == 0)):
            (nc.vector.tensor_copy if even else nc.scalar.copy)(out, in_)

        tp = pst.tile([128, K], f32)
        nc.tensor.matmul(tp[:], BD[:, 128 * g:128 * (g + 1)],
                         Sn[:, K * g:K * (g + 1)],
                         start=True, stop=True)
        cp_b(Tsb[:, K * g:K * (g + 1)], tp[:])

        xt = xts[g // XCH]
        xo = psx.tile([GRP * K, D], f32)
        nc.tensor.matmul(xo[:], SD[:, GRP * K * g:GRP * K * (g + 1)],
                         xt[:, D * (g % XCH):D * (g % XCH + 1)],
                         start=True, stop=True)
        cp_a(XOs[:, D * g:D * (g + 1)], xo[:])

        ao = psa.tile([GRP * K, K], f32)
        nc.tensor.matmul(ao[:], SD[:, GRP * K * g:GRP * K * (g + 1)],
                         Tsb[:, K * g:K * (g + 1)],
                         start=True, stop=True)
        cp_b(AOs[:, K * g:K * (g + 1)], ao[:])

        # ship finished XO chunks on the scalar ring
        if g % OCH == OCH - 1:
            c = g // OCH
            nc.scalar.dma_start(
                XO_d[GRP * K * OCH * c:GRP * K * OCH * (c + 1), :].rearrange(
                    "(g r) d -> r g d", r=GRP * K),
                XOs[:, D * OCH * c:D * OCH * (c + 1)].rearrange(
                    "r (g d) -> r g d", d=D),
            )
        # ship AO halves as they complete
        if g in (NG // 2 - 1, NG - 1):
            h = 0 if g == NG // 2 - 1 else 1
            hw = NG // 2 * K
            nc.scalar.dma_start(
                AO_d[GPC * K // 2 * h:GPC * K // 2 * (h + 1), :].rearrange(
                    "(g r) k -> r g k", r=GRP * K),
                AOs[:, hw * h:hw * (h + 1)].rearrange("r (g k) -> r g k", k=K),
            )



def _build():
    if "nc" in _CACHE:
        return _CACHE["nc"]
    import concourse.bacc as bacc
    import concourse.tile as tile
    import concourse.mybir as mybir

    f32 = mybir.dt.float32
    nc = bacc.Bacc("TRN2", target_bir_lowering=False, debug=False)
    S_d = nc.dram_tensor("S", [ROWS, K], f32, kind="ExternalInput").ap()
    X_d = nc.dram_tensor("X", [ROWS, D], mybir.dt.float32r, kind="ExternalInput").ap()
    AT_d = nc.dram_tensor("AT", [ROWS, NPER], f32, kind="ExternalInput").ap()
    XO_d = nc.dram_tensor("XO", [GPC * K, D], f32, kind="ExternalOutput").ap()
    AO_d = nc.dram_tensor("AO", [GPC * K, K], f32, kind="ExternalOutput").ap()

    with tile.TileContext(nc) as tc:
        with ExitStack() as ctx:
            _body(ctx, tc, S_d, X_d, AT_d, XO_d, AO_d)
    nc.compile()
    _CACHE["nc"] = nc
    return nc


def kernel(S, A, X, idx=None, n=NPER, **_):
    global LAST_RESULTS
    from concourse.bass_utils import run_bass_kernel_spmd

    S = np.ascontiguousarray(np.asarray(S, dtype=np.float32))
    A = np.asarray(A, dtype=np.float32)
    X = np.ascontiguousarray(np.asarray(X, dtype=np.float32))
    n = int(np.asarray(n)) if n is not None else NPER
    assert n == NPER and S.shape == (B * NPER, K) and X.shape == (B * NPER, D)

    # Diagonal 32x32 blocks of A, transposed per block: AT[32j+q, p] = A_j[p, q]
    bi = np.arange(B)
    blocks = A.reshape(B, NPER, B, NPER)[bi, :, bi, :]        # [384, 32, 32]
    AT = np.ascontiguousarray(blocks.transpose(0, 2, 1)).reshape(B * NPER, NPER)

    S8 = S.reshape(NCORES, ROWS, K)
    X8 = X.reshape(NCORES, ROWS, D)
    AT8 = AT.reshape(NCORES, ROWS, NPER)
    in_maps = [{"S": S8[c], "X": X8[c], "AT": AT8[c]} for c in range(NCORES)]

    nc = _build()
    kw = {}
    if os.environ.get("KERNEL_TRACE"):
        kw = dict(trace=True, tmpdir=os.environ.get("KERNEL_TRACE_DIR") or None)
    res = run_bass_kernel_spmd(nc, in_maps, list(range(NCORES)), **kw)
    LAST_RESULTS = res

    X_cat = np.concatenate([r["XO"] for r in res.results], axis=0)   # [3840,300]
    AO = np.stack([r["AO"] for r in res.results]).reshape(B, K, K)
    A_bd = np.zeros((B * K, B * K), dtype=np.float32)
    A_bd.reshape(B, K, B, K)[bi, :, bi, :] = AO
    return X_cat, A_bd


# revision 12
# speedup vs baseline: 1.5874x; 1.1463x over previous
"""DiffPool encoder kernel for Trainium2 (Bass/Tile), 8-core SPMD.

Problem (hardcoded shapes):
  S [12288, 10] f32 assignment logits, A [12288, 12288] f32 adjacency,
  X [12288, 300] f32 features, idx [12288] i64 (sorted graph ids),
  n () i64 = 32 nodes/graph. 384 graphs.

  out0 X_cat [3840, 300] = concat_g softmax(S_g)^T X_g
  out1 A_bd [3840, 3840] = block_diag_g softmax(S_g)^T A_g softmax(S_g)

Sharding: graphs split across 8 cores (48 graphs each). Pooling is
block-diagonal per graph, so each core only needs its rows of S/X and the
48 diagonal 32x32 blocks of A. Per core, graphs run in 12 groups of 4
(4*32 = 128 nodes = full partition dim):
  - softmax over K=10 for all 1536 rows (one exp + segmented reduce)
  - SD [128, 12*40]: per group a block-diag [128,40] of normalized S
  - BD [128, 12*128]: per group a block-diag [128,128] of A_b^T
  - mm2: T  = BD_g^T @ Sn_g = A_b @ softmax(S_b), stacked    [128,10]
  - mm1: XO = SD_g^T @ X_g                                   [40,300]
  - mm3: AO = SD_g^T @ T_g                                   [40,10]

The host pre-arranges every input into the exact per-core SBUF layout so
each load is one flat 2D DMA (contiguous per partition; no 40-byte
scatter packets), and post-rearranges the device-layout outputs.

Engine/queue routing: sync HWDGE ring = AT + X chunks 0,2 + all stores;
scalar HWDGE ring = S + X chunks 1,3.  DVE runs BD placement before the
softmax reduce chain so the PE can start mm2s early; PSUM evacuation
alternates ACT/DVE.  Matmul dtype is fp32 by default (exact, 2-pass PE);
set KERNEL_F32R=1 for single-pass float32r (~4x faster mm1, ~2e-4 error).
"""

import os
import numpy as np
from contextlib import ExitStack

B = 384        # graphs
NPER = 32      # nodes per graph
K = 10         # clusters
D = 300        # feature dim
NCORES = 8
GPC = B // NCORES          # 48 graphs per core
GRP = 4                    # graphs per 128-row group
NG = GPC // GRP            # 12 groups per core
ROWS = GPC * NPER          # 1536 node rows per core
XCH = 3                    # groups per X input DMA chunk
OCH = 3                    # groups per XO output DMA chunk

_CACHE = {}
LAST_RESULTS = None        # BassKernelResults of the most recent run


def _use_f32r():
    return bool(os.environ.get("KERNEL_F32R"))


def _body(ctx, tc, S_d, X_d, AT_d, XO_d, AO_d):
    import concourse.bass as bass
    import concourse.mybir as mybir

    nc = tc.nc
    f32 = mybir.dt.float32
    mmdt = mybir.dt.float32r if _use_f32r() else f32

    const = ctx.enter_context(tc.tile_pool(name="const", bufs=1))
    psx = ctx.enter_context(tc.tile_pool(name="psx", bufs=3, space="PSUM"))
    psa = ctx.enter_context(tc.tile_pool(name="psa", bufs=2, space="PSUM"))
    pst = ctx.enter_context(tc.tile_pool(name="pst", bufs=2, space="PSUM"))

    S_t = const.tile([128, NG * K], f32, tag="S_t")
    E = const.tile([128, NG * K], f32, tag="E")
    Sn = const.tile([128, NG * K], mmdt, tag="Sn")
    sums = const.tile([128, NG], f32, tag="sums")
    rinv = const.tile([128, NG], f32, tag="rinv")
    Ast = const.tile([128, NG * NPER], f32, tag="Ast")
    SD = const.tile([128, NG * GRP * K], mmdt, tag="SD")
    BD = const.tile([128, NG * 128], mmdt, tag="BD")
    Tsb = const.tile([128, NG * K], mmdt, tag="Tsb")
    X_all = const.tile([128, NG * D], mmdt, tag="X_all")
    XOs = const.tile([GRP * K, NG * D], f32, tag="XOs")
    AOs = const.tile([GRP * K, NG * K], f32, tag="AOs")

    # ---- zero-fills on GpSimd (f32 view), off every critical chain ----
    nc.gpsimd.memset(SD[:].bitcast(f32), 0.0)
    nc.gpsimd.memset(BD[:].bitcast(f32), 0.0)

    # ---- inputs: all flat 2D DMAs (host pre-arranged layouts) ----
    nc.scalar.dma_start(S_t[:], S_d)
    nc.sync.dma_start(Ast[:], AT_d)
    NCH = NG // XCH
    for c in range(NCH):
        w = XCH * D
        (nc.sync if c % 2 == 0 else nc.scalar).dma_start(
            X_all[:, w * c:w * (c + 1)], X_d[:, w * c:w * (c + 1)])

    # ---- block-diag A^T placement on DVE first (only needs AT) ----
    A3 = Ast[:].rearrange("p (g q) -> p g q", q=NPER)
    BDv = BD[:].rearrange("p (g m) -> p g m", m=128)
    for b in range(GRP):
        ps = slice(NPER * b, NPER * (b + 1))
        nc.vector.tensor_copy(BDv[ps, :, NPER * b:NPER * (b + 1)], A3[ps, :, :])

    # ---- softmax over K within each group column block ----
    nc.scalar.activation(E[:], S_t[:], mybir.ActivationFunctionType.Exp)
    E3 = E[:].rearrange("p (g k) -> p g k", k=K)
    nc.vector.reduce_sum(sums[:], E3, axis=mybir.AxisListType.X)
    nc.vector.reciprocal(rinv[:], sums[:])
    rb = rinv[:].unsqueeze(2)
    nc.vector.tensor_mul(Sn[:].rearrange("p (g k) -> p g k", k=K), E3,
                         rb.broadcast_to([128, NG, K]))

    # ---- block-diag softmax placement on DVE ----
    Sn3 = Sn[:].rearrange("p (g k) -> p g k", k=K)
    SDv = SD[:].rearrange("p (g m) -> p g m", m=GRP * K)
    for b in range(GRP):
        ps = slice(NPER * b, NPER * (b + 1))
        nc.vector.tensor_copy(SDv[ps, :, K * b:K * (b + 1)], Sn3[ps, :, :])

    # ---- loop A: all mm2 first (PE can start as soon as BD+Sn land) ----
    for g in range(NG):
        tp = pst.tile([128, K], f32)
        nc.tensor.matmul(tp[:], BD[:, 128 * g:128 * (g + 1)],
                         Sn[:, K * g:K * (g + 1)], start=True, stop=True)
        nc.scalar.copy(Tsb[:, K * g:K * (g + 1)], tp[:])

    # ---- loop B: mm1 + mm3 per group, stores on the sync ring ----
    for g in range(NG):
        def cp_a(out, in_, even=(g % 2 == 0)):
            (nc.scalar.copy if even else nc.vector.tensor_copy)(out, in_)

        def cp_b(out, in_, even=(g % 2 == 0)):
            (nc.vector.tensor_copy if even else nc.scalar.copy)(out, in_)

        xo = psx.tile([GRP * K, D], f32)
        nc.tensor.matmul(xo[:], SD[:, GRP * K * g:GRP * K * (g + 1)],
                         X_all[:, D * g:D * (g + 1)], start=True, stop=True)
        cp_a(XOs[:, D * g:D * (g + 1)], xo[:])

        ao = psa.tile([GRP * K, K], f32)
        nc.tensor.matmul(ao[:], SD[:, GRP * K * g:GRP * K * (g + 1)],
                         Tsb[:, K * g:K * (g + 1)], start=True, stop=True)
        cp_b(AOs[:, K * g:K * (g + 1)], ao[:])

        if g % OCH == OCH - 1:
            c = g // OCH
            nc.sync.dma_start(XO_d[:, D * OCH * c:D * OCH * (c + 1)],
                              XOs[:, D * OCH * c:D * OCH * (c + 1)])
        if g in (NG // 2 - 1, NG - 1):
            h = 0 if g == NG // 2 - 1 else 1
            hw = NG // 2 * K
            nc.sync.dma_start(AO_d[:, hw * h:hw * (h + 1)],
                              AOs[:, hw * h:hw * (h + 1)])


def _build():
    key = ("nc", _use_f32r())
    if key in _CACHE:
        return _CACHE[key]
    import concourse.bacc as bacc
    import concourse.tile as tile
    import concourse.mybir as mybir

    f32 = mybir.dt.float32
    mmdt = mybir.dt.float32r if _use_f32r() else f32
    nc = bacc.Bacc("TRN2", target_bir_lowering=False, debug=False)
    # Device-layout tensors (host pre/post-arranges):
    #   S  [128, 120]   col = 10g + k, partition = node p of group g
    #   AT [128, 384]   [32b+q, 32g+p] = A_{4g+b}[p, q]
    #   X  [128, 3600]  col = 300g + d
    #   XO [40, 3600]   row = 10b + i, col = 300g + d  (graph j = 4g+b)
    #   AO [40, 120]    row = 10b + i, col = 10g + k
    S_d = nc.dram_tensor("S", [128, NG * K], f32, kind="ExternalInput").ap()
    X_d = nc.dram_tensor("X", [128, NG * D], mmdt, kind="ExternalInput").ap()
    AT_d = nc.dram_tensor("AT", [128, NG * NPER], f32, kind="ExternalInput").ap()
    XO_d = nc.dram_tensor("XO", [GRP * K, NG * D], f32, kind="ExternalOutput").ap()
    AO_d = nc.dram_tensor("AO", [GRP * K, NG * K], f32, kind="ExternalOutput").ap()

    with tile.TileContext(nc) as tc:
        with ExitStack() as ctx:
            _body(ctx, tc, S_d, X_d, AT_d, XO_d, AO_d)
    nc.compile()
    _CACHE[key] = nc
    return nc


def kernel(S, A, X, idx=None, n=NPER, **_):
    global LAST_RESULTS
    from concourse.bass_utils import run_bass_kernel_spmd

    S = np.asarray(S, dtype=np.float32)
    A = np.asarray(A, dtype=np.float32)
    X = np.asarray(X, dtype=np.float32)
    n = int(np.asarray(n)) if n is not None else NPER
    assert n == NPER and S.shape == (B * NPER, K) and X.shape == (B * NPER, D)

    # Device layouts (see _build).  c = core, g = group, b = graph-in-group.
    S8 = np.ascontiguousarray(
        S.reshape(NCORES, NG, 128, K).transpose(0, 2, 1, 3)
    ).reshape(NCORES, 128, NG * K)
    X8 = np.ascontiguousarray(
        X.reshape(NCORES, NG, 128, D).transpose(0, 2, 1, 3)
    ).reshape(NCORES, 128, NG * D)
    bi = np.arange(B)
    blocks = A.reshape(B, NPER, B, NPER)[bi, :, bi, :]        # [384, 32, 32]
    blocksT = blocks.transpose(0, 2, 1)                       # [j][q, p] = A_j[p, q]
    AT8 = np.ascontiguousarray(
        blocksT.reshape(NCORES, NG, GRP, NPER, NPER).transpose(0, 2, 3, 1, 4)
    ).reshape(NCORES, 128, NG * NPER)

    in_maps = [{"S": S8[c], "X": X8[c], "AT": AT8[c]} for c in range(NCORES)]

    nc = _build()
    kw = {}
    if os.environ.get("KERNEL_TRACE"):
        kw = dict(trace=True, tmpdir=os.environ.get("KERNEL_TRACE_DIR") or None)
    res = run_bass_kernel_spmd(nc, in_maps, list(range(NCORES)), **kw)
    LAST_RESULTS = res

    # XO [40, 3600] -> per-core [12, 40, 300] -> rows 40g+10b+i of X_cat
    X_cat = np.concatenate(
        [r["XO"].reshape(GRP * K, NG, D).transpose(1, 0, 2).reshape(GPC * K, D)
         for r in res.results], axis=0)
    # AO [40, 120] -> blocks [g, b][i, k] -> graph j = 4g+b
    AO = np.stack(
        [r["AO"].reshape(GRP, K, NG, K).transpose(2, 0, 1, 3).reshape(GPC, K, K)
         for r in res.results]).reshape(B, K, K)
    A_bd = np.zeros((B * K, B * K), dtype=np.float32)
    A_bd.reshape(B, K, B, K)[bi, :, bi, :] = AO
    return X_cat, A_bd
